# revision 14
# baseline (speedup 1.0000x reference)
"""Trainium2 Bass kernel for the BDH-style recurrent block.

Strategy: data-parallel over B (8 batches -> 8 NeuronCores, no collectives).
The T=128-step scan is de-sequentialized into dense matmuls per core:

  u_t = relu(emb_t @ Dx.T)                                  (T,N)
  x_t = (XD*x_{t-1} + u_t)/s_t  with s_t = XD + sum(u_t)    (L1 norm; x>=0)
      => x = C @ u, C[t,s] = (1/s_s) exp(A_t - A_s), A_t = cumsum log(XD/s_r)
  G   = X X^T = C (u u^T) C^T       <-- Gram from u directly: the u-gram
        Ghat = u u^T is C-independent, so it overlaps the serial C-phase
  a*_t = ((DecayMask . G) @ ln(emb))_t                      (rho_0 = 0)
  y_t  = relu(ln(a*_t) @ Dy.T) * x_t                        (x_t >= 0)
  v*_t = ln(y_t @ E.T)

Perf model (TRN2): PE streams 1 col/cycle for fp16 at any free size, but
f32r gets a 4x penalty below 256 free; the PE clock ramps 1.2->2.4 GHz only
after ~3us of continuous execution, so the instruction stream is packed to
avoid PE idle gaps (warmup on the first-arriving DMA block, dependency-gated
keepalive matmuls across serial LN/scalar stretches).  All matmul operands
are fp16 (weights cast on host: halves HBM traffic), accumulation stays
f32 in PSUM.  Layernorm rstd uses exp(-0.5*ln(var+eps)) so the scalar
engine needs exactly one activation table (natural_log_exp_and_others) --
no 1.3us mid-kernel table reloads.  PSUM evacuations rotate across the
scalar/vector/gpsimd engines.
"""

import math
from contextlib import ExitStack

import numpy as np

N = 2048
D = 256
B = 8
T = 128
XD = 0.97
UD = 0.97
LN_EPS = 1e-5
L1_EPS = 1e-12

# log-domain recentring: E[sum relu(N(0,1)) over 2048] + XD ~ 818.9
LNC2INV = 6.7065
C2 = math.exp(-LNC2INV)
K1 = LNC2INV - math.log(XD)

KD = D // 128   # 2
KN = N // 128   # 16
NJ = N // 512   # 4
WARMUP_MMS = 9

# head16 fp16 block: [ident(T) | embT(KD*T) | emb(D)]
H16_IDENT = 0
H16_EMBT = T
H16_EMB = T + KD * T
H16_COLS = T + KD * T + D
# head32 f32 block: [trik(T) | dmaskT(T) | xdvec(1) | utones(T) | emb(D)]
H32_TRIK = 0
H32_DMASKT = T
H32_XDVEC = 2 * T
H32_UTONES = 2 * T + 1
H32_EMB = 3 * T + 1
H32_COLS = 3 * T + 1 + D

_cache = {}


def _pack_jk(wT):
    # (KD,128,N) k-major -> (128, [j(4), k(2), 512]) per-partition contiguous
    return np.ascontiguousarray(
        wT.reshape(KD, 128, NJ, 512).transpose(1, 2, 0, 3).reshape(128, KD * N))


def _consts32():
    r = np.arange(T)
    tri = r[None, :] - r[:, None]                                   # t - s
    trik = np.where(tri >= 0, -K1 * tri - LNC2INV, -10000.0).astype(np.float32)
    pw = r[:, None] - 1 - r[None, :]                                # [t,s] t-1-s
    dmask = np.where(pw >= 0, UD ** np.maximum(pw, 0), 0.0).astype(np.float32)
    dmaskT = np.ascontiguousarray(dmask.T)                          # [s,t]
    xdvec = np.full((T, 1), C2 * XD, dtype=np.float32)
    xdvec[0, 0] = 0.0                                               # x_{-1} = 0
    utones = (r[:, None] <= r[None, :]).astype(np.float32)          # [r,t] r<=t
    return np.ascontiguousarray(np.concatenate(
        [trik, dmaskT, xdvec, utones], axis=1))


def _split_multiwait(nc, mybir):
    """This walrus build caps sync waits per instruction (1 for regular
    instructions, 2 for EventSemaphore). Tile attaches more (e.g. the
    kernel-tail Drain waits on every live semaphore). Hoist excess waits
    onto same-engine NOPs placed immediately before the instruction —
    engine queues are sequential, so semantics are preserved."""
    n = 0
    for f in nc.m.functions:
        for bb in f.blocks:
            out = []
            changed = False
            for ins in bb.instructions:
                si = ins.sync_info
                ow = list(si.on_wait) if si is not None else []
                cap = 2 if ins.opcode == "EventSemaphore" else 1
                if len(ow) > cap:
                    sem_waits = [w for w in ow if w.sync_type == "semaphore"]
                    other = [w for w in ow if w.sync_type != "semaphore"]
                    keep = max(cap - len(other), 0)
                    hoist = sem_waits[:len(sem_waits) - keep] if keep else sem_waits
                    kept = sem_waits[len(hoist):] + other
                    assert len(kept) <= cap, (len(kept), cap, ins.opcode)
                    changed = True
                    for w in hoist:
                        n += 1
                        nop = mybir.InstNoOp(
                            name=f"wsplit-{n}",
                            sync_info=mybir.SyncInfo(on_wait=[w], on_update=[]),
                            bass_nofuse=True,
                            engine=ins.engine,
                        )
                        nc.register_instruction(nop, overwrite=True)
                        out.append(nop)
                    si.on_wait = kept
                out.append(ins)
            if changed:
                bb.instructions = out
    return nc


def _build():
    import concourse.bass as bass
    import concourse.mybir as mybir
    import concourse.tile as tile

    f32 = mybir.dt.float32
    f16 = mybir.dt.float16
    AF = mybir.ActivationFunctionType
    ALU = mybir.AluOpType
    AX = mybir.AxisListType

    from concourse.vector_clock import ScopedClock

    class _TrimTailTC(tile.TileContext):
        # Drop the second kernel-tail all-engine barrier: it only orders
        # the semaphore resets against engine halt, and nothing executes
        # after it. The first barrier (before resets) is kept, so resets
        # still happen on a quiesced machine and re-execution stays safe.
        def _drain_and_barrier(self, tick_clock, wait_clock):
            drain_inst = self.nc.sync.drain()
            wait_clock.add_sem_waits(
                drain_inst.ins, ScopedClock({None: tick_clock.global_clock})
            )
            self.nc.all_engine_barrier()
            assert self.sems is not None
            popped = self.nc._tile_sem_poison_stack.pop()
            assert popped is self._sem_poison
            self.nc.clear_and_free_semaphores(
                list(self.sems.allocated().values())
            )

    nc = bass.Bass()

    d_h16 = nc.dram_tensor("h16", [128, H16_COLS], f16, kind="ExternalInput")
    d_h32 = nc.dram_tensor("h32", [128, H32_COLS], f32, kind="ExternalInput")
    d_dxT = nc.dram_tensor("dxT", [128, KD * N], f16, kind="ExternalInput")
    d_dyT = nc.dram_tensor("dyT", [128, KD * N], f16, kind="ExternalInput")
    d_eT = nc.dram_tensor("eT", [128, KN * D], f16, kind="ExternalInput")
    d_out = nc.dram_tensor("out", [T, D], f32, kind="ExternalOutput")

    with _TrimTailTC(nc) as tc, ExitStack() as ctx:
        work = ctx.enter_context(tc.tile_pool(name="work", bufs=1))
        stats = ctx.enter_context(tc.tile_pool(name="stats", bufs=1))
        p_u = ctx.enter_context(tc.tile_pool(name="p_u", bufs=2, space="PSUM"))
        p_sq = ctx.enter_context(tc.tile_pool(name="p_sq", bufs=4, space="PSUM"))
        p_g = ctx.enter_context(tc.tile_pool(name="p_g", bufs=1, space="PSUM"))
        p_med = ctx.enter_context(tc.tile_pool(name="p_med", bufs=1, space="PSUM"))

        # ---- activation table preload: Ln+Exp+Square+Relu+Copy+Identity all
        # live in the natural_log_exp_and_others table -> exactly one load.
        pre_sb = stats.tile([1, 1], f32)
        nc.vector.memset(pre_sb[:], 1.0)
        pre_o = stats.tile([1, 1], f32)
        nc.scalar.activation(pre_o[:], pre_sb[:], AF.Ln)
        nc.scalar.activation(pre_o[:], pre_sb[:], AF.Exp)

        negrow = stats.tile([1, T], f32)
        nc.gpsimd.memset(negrow[:], -1.0)

        # ---- DMAs: issue order = delivery order on the qSP-HWDGE FIFO.
        h16_sb = work.tile([128, H16_COLS], f16)
        nc.sync.dma_start(h16_sb[:], d_h16[:])
        ident16 = h16_sb[:, H16_IDENT:H16_IDENT + T]
        embT_sb = h16_sb[:, H16_EMBT:H16_EMBT + KD * T]
        emb16_sb = h16_sb[:, H16_EMB:H16_EMB + D]

        # 2 halves: 4KB-per-partition descriptors keep the SDMA engines at
        # full rate (2KB descriptors measurably halve throughput).
        dxT_sb = work.tile([128, KD * N], f16)
        for h in range(2):
            nc.sync.dma_start(dxT_sb[:, h * 2048:(h + 1) * 2048],
                              d_dxT[:, h * 2048:(h + 1) * 2048])
        h32_sb = work.tile([128, H32_COLS], f32)
        nc.sync.dma_start(h32_sb[:], d_h32[:])
        trik_sb = h32_sb[:, H32_TRIK:H32_TRIK + T]
        dmaskT_sb = h32_sb[:, H32_DMASKT:H32_DMASKT + T]
        xdvec_sb = h32_sb[:, H32_XDVEC:H32_XDVEC + 1]
        utones_sb = h32_sb[:, H32_UTONES:H32_UTONES + T]
        emb_sb = h32_sb[:, H32_EMB:H32_EMB + D]

        dyT_sb = work.tile([128, KD * N], f16)
        nc.sync.dma_start(dyT_sb[:], d_dyT[:])
        eT_sb = work.tile([128, KN * D], f16)
        nc.sync.dma_start(eT_sb[:], d_eT[:])

        # ---- PE warmup on locally-generated random data (no DMA wait);
        # keeps the PE clock ramping until the first dxT piece arrives.
        rng_sb = work.tile([128, 512], f16)
        nc.vector.random(rng_sb[:])
        wu_ps = p_u.tile([128, 512], f32, tag="pu")
        for _ in range(WARMUP_MMS):
            nc.tensor.matmul(wu_ps[:], rng_sb[:, 0:T], rng_sb[:],
                             start=True, stop=True)

        def keepalive(ap, cols=T):
            # PE matmuls gated on a late stat: hold the clock-ramp warm
            # through serial non-PE stretches.
            ka = p_sq.tile([T, T], f32, tag="sq")
            nc.tensor.matmul(ka[0:1, 0:cols], ap, trik_sb[:, 0:cols],
                             start=True, stop=True)

        def fast_ln(src, dst, tagp, hold_pe=False):
            """dst = LN(src) over free dim; rstd = exp(-0.5*ln(var+eps)) so
            no Sqrt table is ever loaded."""
            stat6 = stats.tile([T, 6], f32, tag=f"{tagp}_s6")
            nc.vector.bn_stats(stat6[:], src)
            mv = stats.tile([T, 2], f32, tag=f"{tagp}_mv")
            nc.vector.bn_aggr(mv[:], stat6[:])
            if hold_pe:
                keepalive(mv[:, 0:1])
            veps = stats.tile([T, 1], f32, tag=f"{tagp}_ve")
            nc.vector.tensor_scalar_add(veps[:], mv[:, 1:2], LN_EPS)
            lv = stats.tile([T, 1], f32, tag=f"{tagp}_lv")
            nc.scalar.activation(lv[:], veps[:], AF.Ln)
            rstd = stats.tile([T, 1], f32, tag=f"{tagp}_rs")
            nc.scalar.activation(rstd[:], lv[:], AF.Exp, scale=-0.5)
            if hold_pe:
                keepalive(rstd[:])
            nmr = stats.tile([T, 1], f32, tag=f"{tagp}_nr")
            nc.vector.scalar_tensor_tensor(nmr[:], mv[:, 0:1], -1.0, rstd[:],
                                           op0=ALU.mult, op1=ALU.mult)
            nc.scalar.activation(dst[:], src, AF.Identity,
                                 scale=rstd[:], bias=nmr[:])

        # ---- u = relu(emb @ Dx.T), fp16, rowsums; evacs rotate engines ----
        u_sb = work.tile([T, N], f16)
        su_part = stats.tile([T, 2 * NJ], f32)
        ut_sb = work.tile([128, N], f16)
        g_ps = p_g.tile([T, T], f32, tag="g")

        def _emit_tp(j):
            # u^T chunks via PE transpose (fp16); evacs rotate DVE/Pool
            for cc in range(4):
                c = 4 * j + cc
                tp = p_sq.tile([T, T], f16, tag="sq")
                nc.tensor.transpose(tp[:], u_sb[:, c * T:(c + 1) * T], ident16)
                if c % 2 == 0:
                    nc.vector.tensor_copy(ut_sb[:, c * T:(c + 1) * T], tp[:])
                else:
                    nc.scalar.copy(ut_sb[:, c * T:(c + 1) * T], tp[:])

        def _emit_gram(j):
            # Ghat += uT_c uT_c^T, accumulated across all 16 chunks
            for cc in range(4):
                c = 4 * j + cc
                nc.tensor.matmul(g_ps[:], ut_sb[:, c * T:(c + 1) * T],
                                 ut_sb[:, c * T:(c + 1) * T],
                                 start=(c == 0), stop=(c == KN - 1))

        for j in range(NJ):
            ps = p_u.tile([128, 512], f32, tag="pu")
            for c in range(KD):
                nc.tensor.matmul(
                    ps[:],
                    embT_sb[:, c * T:(c + 1) * T],
                    dxT_sb[:, j * 1024 + c * 512: j * 1024 + (c + 1) * 512],
                    start=(c == 0),
                    stop=(c == KD - 1),
                )
            # halves evac in parallel on ACT and DVE: su arrives sooner
            nc.scalar.activation(u_sb[:, j * 512:j * 512 + 256], ps[:, 0:256],
                                 AF.Relu, accum_out=su_part[:, 2 * j:2 * j + 1])
            nc.vector.tensor_scalar(u_sb[:, j * 512 + 256:(j + 1) * 512],
                                    ps[:, 256:512], 0.0, 0.0, op0=ALU.max,
                                    op1=ALU.add,
                                    accum_out=su_part[:, 2 * j + 1:2 * j + 2])

            # transposes of the previous piece's chunks fill the DMA-wait
            # gaps on the PE queue without delaying the next u matmul.
            if j >= 1:
                _emit_tp(j - 1)
            if j >= 2:
                _emit_gram(j - 2)

        # ---- C^T coefficient chain emitted FIRST (critical path): the
        # remaining transposes/grams fill the PE behind it.
        su = stats.tile([T, 1], f32)
        nc.vector.tensor_reduce(su[:], su_part[:], axis=AX.X, op=ALU.add)
        keepalive(su[:])
        q_sb = stats.tile([T, 1], f32)
        nc.scalar.activation(q_sb[:], su[:], AF.Ln, scale=C2, bias=xdvec_sb)

        qc = p_sq.tile([T, T], f32, tag="sq")               # Q_s column
        nc.tensor.matmul(qc[:, 0:1], utones_sb, q_sb[:], start=True, stop=True)
        qr = p_sq.tile([T, T], f32, tag="sq")               # Q_t row
        nc.tensor.matmul(qr[0:1, :], q_sb[:], utones_sb, start=True, stop=True)
        qr_sb = stats.tile([1, T], f32)
        nc.vector.tensor_copy(qr_sb[:], qr[0:1, :])
        colsc = stats.tile([T, 1], f32)                     # Q_s - q_s
        nc.vector.tensor_sub(colsc[:], qc[:, 0:1], q_sb[:])
        bc = p_sq.tile([T, T], f32, tag="sq")               # [s,t] = -Q_t
        nc.tensor.matmul(bc[:], negrow[:], qr_sb[:], start=True, stop=True)

        expo = work.tile([T, T], f32)
        nc.vector.scalar_tensor_tensor(
            expo[:], bc[:], colsc[:], trik_sb, op0=ALU.add, op1=ALU.add
        )
        expoc = work.tile([T, T], f32)
        nc.vector.tensor_scalar_max(expoc[:], expo[:], -80.0)
        ct_sb = work.tile([T, T], f16)                      # C^T [s,t]
        nc.scalar.activation(ct_sb[:], expoc[:], AF.Exp)

        _emit_tp(NJ - 1)
        _emit_gram(NJ - 2)
        _emit_gram(NJ - 1)

        # Ghat evac (fp16) for use as stationary in H = Ghat @ C^T
        ghat_sb = work.tile([T, T], f16)
        nc.vector.tensor_copy(ghat_sb[:], g_ps[:])
        keepalive(colsc[:])
        keepalive(expo[:, 0:1])
        keepalive(expoc[:, 0:1])

        # ---- emb row-rstd: LN(a*) is shift-invariant per row, so
        # a* = W @ LN(emb) can use raw emb with rstd folded into W --
        # vn's mean-subtract and normalize apply are never materialized.
        vst6 = stats.tile([T, 6], f32)
        nc.vector.bn_stats(vst6[:], emb_sb)
        vmv = stats.tile([T, 2], f32)
        nc.vector.bn_aggr(vmv[:], vst6[:])
        vveps = stats.tile([T, 1], f32)
        nc.vector.tensor_scalar_add(vveps[:], vmv[:, 1:2], LN_EPS)
        vlv = stats.tile([T, 1], f32)
        nc.scalar.activation(vlv[:], vveps[:], AF.Ln)
        rstd_vn = stats.tile([T, 1], f32)
        nc.scalar.activation(rstd_vn[:], vlv[:], AF.Exp, scale=-0.5)

        # ---- G = C Ghat C^T, W = dmask . G, a* = W @ vn ------------------
        hx = p_sq.tile([T, T], f32, tag="sq")               # H = Ghat @ C^T
        nc.tensor.matmul(hx[:], ghat_sb[:], ct_sb[:], start=True, stop=True)

        # X = C @ u (t,n-major fp16) -- fills PE while H evacs / LN runs
        x_sb = work.tile([T, N], f16)
        x_psums = []
        for j in range(NJ):
            ps = p_u.tile([128, 512], f32, tag="pu")
            nc.tensor.matmul(ps[:], ct_sb[:], u_sb[:, j * 512:(j + 1) * 512],
                             start=True, stop=True)
            if j == 0:
                # slot the H->G chain into the PE queue between X matmuls
                h_sb = work.tile([T, T], f16)
                nc.vector.tensor_copy(h_sb[:], hx[:])
                gm = p_g.tile([T, T], f32, tag="g")
                nc.tensor.matmul(gm[:], ct_sb[:], h_sb[:], start=True,
                                 stop=True)
                # W'[s,t] = Gm[s,t]*rstd_vn[s]*dmaskT[s,t]: the emb
                # row-rstd folds into the mask multiply for free
                wt_sb = work.tile([T, T], f16)
                nc.vector.scalar_tensor_tensor(wt_sb[:], gm[:], rstd_vn[:],
                                               dmaskT_sb, op0=ALU.mult,
                                               op1=ALU.mult)
                aps = p_med.tile([T, D], f32, tag="med")
                nc.tensor.matmul(aps[:], wt_sb[:], emb16_sb, start=True,
                                 stop=True)
            x_psums.append(ps)

        # ---- LN(a*), transpose -------------------------------------------
        lna_sb = work.tile([T, D], f16)
        fast_ln(aps[:], lna_sb, "la", hold_pe=True)

        lnaT_sb = work.tile([128, KD * T], f16)
        for c in range(KD):
            tp = p_sq.tile([T, T], f16, tag="sq")
            nc.tensor.transpose(tp[:], lna_sb[:, c * T:(c + 1) * T], ident16)
            nc.vector.tensor_copy(lnaT_sb[:, c * T:(c + 1) * T], tp[:])

        # X evacs after the LN chain: each is needed only by Y-phase j
        for j in range(NJ):
            dst = x_sb[:, j * 512:(j + 1) * 512]
            if j in (0, 2):
                nc.vector.tensor_copy(dst, x_psums[j][:])
            else:
                nc.scalar.copy(dst, x_psums[j][:])

        # ---- Ycore -> Y -> Y^T -> v_raw, pipelined per j-group -----------
        y_sb = work.tile([T, N], f16)
        y1_sb = work.tile([T, N], f16)
        yt_sb = work.tile([128, N], f16)
        vps = p_med.tile([T, D], f32, tag="med")
        for j in range(NJ):
            ps = p_u.tile([128, 512], f32, tag="pu")
            for k in range(KD):
                nc.tensor.matmul(ps[:], lnaT_sb[:, k * T:(k + 1) * T],
                                 dyT_sb[:, j * 1024 + k * 512: j * 1024 + (k + 1) * 512],
                                 start=(k == 0), stop=(k == KD - 1))
            # relu-evac on ACT, multiply-by-X on GpSimd (SBUF-only engine),
            # yt evacs on DVE: three engines share the Y-phase element work
            nc.scalar.activation(y1_sb[:, j * 512:(j + 1) * 512], ps[:],
                                 AF.Relu)
            nc.gpsimd.tensor_mul(y_sb[:, j * 512:(j + 1) * 512],
                                 y1_sb[:, j * 512:(j + 1) * 512],
                                 x_sb[:, j * 512:(j + 1) * 512])
            for cc in range(4):
                c = 4 * j + cc
                tp = p_sq.tile([T, T], f16, tag="sq")
                nc.tensor.transpose(tp[:], y_sb[:, c * T:(c + 1) * T], ident16)
                nc.vector.tensor_copy(yt_sb[:, c * T:(c + 1) * T], tp[:])
            for cc in range(4):
                c = 4 * j + cc
                nc.tensor.matmul(vps[:], yt_sb[:, c * T:(c + 1) * T],
                                 eT_sb[:, c * D:(c + 1) * D],
                                 start=(c == 0), stop=(c == KN - 1))

        vstar_sb = work.tile([T, D], f32)
        fast_ln(vps[:], vstar_sb, "vs")

        nc.sync.dma_start(d_out[:], vstar_sb[:])

    return _split_multiwait(nc, mybir)


def _numpy_fallback(embeddings, E, Dx, Dy, x_state, rho_state):
    # General-path reference (only used if initial states are nonzero).
    def ln(x):
        m = x.mean(-1, keepdims=True)
        v = ((x - m) ** 2).mean(-1, keepdims=True)
        return (x - m) / np.sqrt(v + LN_EPS)

    x_s = x_state.astype(np.float32).copy()
    rho = rho_state.astype(np.float32).copy()
    outs = np.zeros((B, T, D), dtype=np.float32)
    for t in range(T):
        v_prev = embeddings[:, t, :]
        x_upd = np.maximum(v_prev @ Dx.T, 0.0)
        x_t = XD * x_s + x_upd
        x_t = x_t / np.maximum(np.abs(x_t).sum(-1, keepdims=True), L1_EPS)
        a_star = np.einsum("bdn,bn->bd", rho, x_t)
        y_core = ln(a_star) @ Dy.T
        y_t = np.maximum(y_core, 0.0) * np.maximum(x_t, 0.0)
        outs[:, t, :] = ln(y_t @ E.T)
        vn = ln(v_prev)
        rho = UD * rho + np.einsum("bd,bn->bdn", vn, x_t)
        x_s = x_t
    return outs


def kernel(embeddings, E, Dx, Dy, x_state, rho_state):
    embeddings = np.ascontiguousarray(embeddings, dtype=np.float32)
    E = np.ascontiguousarray(E, dtype=np.float32)
    Dx = np.ascontiguousarray(Dx, dtype=np.float32)
    Dy = np.ascontiguousarray(Dy, dtype=np.float32)

    if np.any(x_state) or np.any(rho_state):
        return _numpy_fallback(embeddings, E, Dx, Dy,
                               np.asarray(x_state, np.float32),
                               np.asarray(rho_state, np.float32))

    from concourse.bass_utils import run_bass_kernel_spmd

    if "nc" not in _cache:
        _cache["nc"] = _build()
    nc = _cache["nc"]

    c32 = _consts32()
    ident16 = np.eye(T, dtype=np.float16)
    dxT = _pack_jk(Dx.T.reshape(KD, 128, N)).astype(np.float16)
    dyT = _pack_jk(Dy.T.reshape(KD, 128, N)).astype(np.float16)
    eT = np.ascontiguousarray(
        E.T.reshape(KN, 128, D).transpose(1, 0, 2).reshape(128, KN * D)
    ).astype(np.float16)

    in_maps = []
    for b in range(B):
        emb_b = embeddings[b]
        embT_b = np.ascontiguousarray(
            emb_b.T.reshape(KD, 128, T).transpose(1, 0, 2).reshape(128, KD * T)
        ).astype(np.float16)
        h16 = np.ascontiguousarray(np.concatenate(
            [ident16, embT_b, emb_b.astype(np.float16)], axis=1))
        h32 = np.ascontiguousarray(np.concatenate([c32, emb_b], axis=1))
        in_maps.append({
            "h16": h16,
            "h32": h32,
            "dxT": dxT,
            "dyT": dyT,
            "eT": eT,
        })

    res = run_bass_kernel_spmd(nc, in_maps, list(range(B)))
    _cache["last_results"] = res
    return np.stack([res.results[i]["out"] for i in range(B)])
